# revision 95
# baseline (speedup 1.0000x reference)
"""Grouped-query attention (B=1, S=2048, HID=4096, 32 q-heads / 8 kv-heads,
D=128, RoPE, additive causal mask) on 8 Trainium2 NeuronCores.

Sharding: tensor-parallel over heads. Core c owns 4 q-heads (columns
512c:512c+512 of Wq), kv-head c (columns 128c:128c+128 of Wk/Wv), and rows
512c:512c+512 of Wo. Each core emits a full-shape fp16 partial of the output
projection; the host sums the 8 partials (the all-reduce of the row-sharded
Wo matmul).

v2 design — all-fp16 data path, fused j-slice pipeline:
  - Everything on the PE runs in fp16 (1 cycle/row at any moving size;
    fp32 accumulation in PSUM), halving all DMA traffic vs fp32.
  - Work is organized as a single pipeline over the four 512-query slices j:
    A(j) projections+RoPE -> B(j) attention -> C(j) output projection.
    B(j)'s inner loop is rate-limited by the scalar engine's exp, so A(j+1)
    and C(j-1) tensor-engine work is interleaved into B(j)'s matmul stream
    as ~850ns filler quanta, keeping the PE gapless (each PE idle gap costs
    ~600ns of p-state penalty on top of the stall).
  - Scores are computed transposed (sc[sk,sq]) so exp output feeds the PV
    matmul directly. The softmax denominator is accumulated on the vector
    engine (Pacc += P_t), reduced+broadcast across partitions by the idle
    GPSIMD engine (partition_all_reduce), avoiding ones-matmuls on the PE.
  - V is projected untransposed (lhsT = x^T tile, rhs = Wv tile) so no
    PE/vector transposes are needed anywhere.
  - Causal structure: off-diagonal score tiles are skipped entirely; the
    four diagonal tiles per (h,j) are computed width-restricted
    (512-128r columns) with a single constant 128x128 additive mask.
  - Host pre-rearranges all inputs partition-major so every DMA is a
    straight wide-descriptor copy.
"""
import os
from collections import deque
from contextlib import ExitStack

import numpy as np

import concourse.bass as bass
import concourse.tile as tile
from concourse import bacc, bass_isa, mybir
from concourse.bass_utils import run_bass_kernel_spmd

F32 = mybir.dt.float32
F16 = mybir.dt.float16
EXP = mybir.ActivationFunctionType.Exp

S = 2048
HID = 4096
D = 128
NCORES = 8
NHQ = 4                      # q heads per core
SCALE = float(D) ** -0.5
SL = 4                       # 512-query j slices
KT = HID // 128              # 32 hidden contraction tiles
NO = HID // 512              # 8 output column slices

_NC_CACHE = {}


def build_nc():
    nc = bacc.Bacc("TRN2", target_bir_lowering=False, debug=False,
                   num_devices=NCORES)
    x_r = nc.dram_tensor("x_r", [128, KT, S], F16, kind="ExternalInput").ap()
    # packed projection weights: cols 0:512 = Wq, 512:640 = Wk, 640:768 = Wv
    w_r = nc.dram_tensor("w_r", [128, KT, 768], F16, kind="ExternalInput").ap()
    wo_r = nc.dram_tensor("wo_r", [128, NHQ, HID], F16, kind="ExternalInput").ap()
    cos_r = nc.dram_tensor("cos_r", [128, S], F16, kind="ExternalInput").ap()
    sinf_r = nc.dram_tensor("sinf_r", [128, S], F16, kind="ExternalInput").ap()
    mask_r = nc.dram_tensor("mask_r", [128, 128], F32, kind="ExternalInput").ap()
    y = nc.dram_tensor("y", [S, HID], F16, kind="ExternalOutput").ap()

    with tile.TileContext(nc) as tc, ExitStack() as ctx, \
            nc.allow_low_precision(reason="fp16 data path validated vs 2e-2 gate"):
        const = ctx.enter_context(tc.tile_pool(name="const", bufs=1))
        xap = ctx.enter_context(tc.tile_pool(name="xap", bufs=8))
        qkp = ctx.enter_context(tc.tile_pool(name="qkp", bufs=2))
        vp = ctx.enter_context(tc.tile_pool(name="vp", bufs=1))
        otp = ctx.enter_context(tc.tile_pool(name="otp", bufs=2))
        tmp = ctx.enter_context(tc.tile_pool(name="tmp", bufs=2))
        ptbp = ctx.enter_context(tc.tile_pool(name="ptbp", bufs=10))
        paccp = ctx.enter_context(tc.tile_pool(name="paccp", bufs=2))
        nrmp = ctx.enter_context(tc.tile_pool(name="nrmp", bufs=2))
        ysp = ctx.enter_context(tc.tile_pool(name="ysp", bufs=2))

        w_sb = const.tile([128, KT, 768], F16)
        wq_sb = w_sb[:, :, 0:512]
        wk_sb = w_sb[:, :, 512:640]
        wv_sb = w_sb[:, :, 640:768]
        wo_sb = const.tile([128, NHQ, HID], F16)
        cos_sb = const.tile([128, S], F16)
        sinf_sb = const.tile([128, S], F16)
        mask_sb = const.tile([128, 128], F32)

        xa = {}      # (j, g) -> [128, 8, 512] x^T quarter-slice
        qts = {}     # (j, h) -> [128, 512] roped Q^T slice
        kts = {}     # j -> [128, 512] roped K^T slice
        vsb = {}     # j -> [128, 512] V slice ([s-in-block, d] layout)
        ots = {}     # (j, h) -> [128, 512] normalized attention out^T

        def issue_x(j):
            for g in range(4):
                t = xap.tile([128, 8, 512], F16, tag="xa", name=f"xa{j}{g}")
                xa[(j, g)] = t
                nc.sync.dma_start(out=t[:], in_=x_r[:, 8 * g:8 * g + 8,
                                                    512 * j:512 * (j + 1)])

        # DMA priority order: per-g weight chunks interleaved with x(0) so the
        # k-outer A(0) can start after the first chunk group; then the rest.
        # The first group is split in half so compute starts ~3.5us sooner.
        for g in range(4):
            subs = ([slice(0, 1), slice(1, 2), slice(2, 4), slice(4, 6),
                     slice(6, 8)] if g == 0
                    else [slice(8 * g, 8 * g + 4), slice(8 * g + 4, 8 * g + 8)])
            t = xap.tile([128, 8, 512], F16, tag="xa", name=f"xa0{g}")
            xa[(0, g)] = t
            for gs in subs:
                nc.sync.dma_start(out=w_sb[:, gs, :], in_=w_r[:, gs, :])
                ls = slice(gs.start - 8 * g, gs.stop - 8 * g)
                nc.sync.dma_start(out=t[:, ls, :], in_=x_r[:, gs, 0:512])
            if g == 1:
                # mid-stream so they land before A(0)'s rope drains, without
                # delaying either the first chunks or x(1).
                nc.sync.dma_start(out=cos_sb[:], in_=cos_r[:])
                nc.sync.dma_start(out=sinf_sb[:], in_=sinf_r[:])
        issue_x(1)
        nc.sync.dma_start(out=mask_sb[:], in_=mask_r[:])
        nc.sync.dma_start(out=wo_sb[:], in_=wo_r[:])

        def rope_drain(ps, j, dst):
            """dst = ps*cos + rotate_half(ps)*sin for slice j (sin sign-folded)."""
            raw = tmp.tile([128, 512], F16, tag="raw", name="raw", bufs=5)
            nc.scalar.copy(raw[:], ps[:])
            cs = cos_sb[:, 512 * j:512 * (j + 1)]
            sn = sinf_sb[:, 512 * j:512 * (j + 1)]
            rot = tmp.tile([128, 512], F16, tag="rot", name="rot", bufs=3)
            nc.vector.tensor_mul(dst[:], raw[:], cs)
            # sin's halves are identical, so with the sign fold on the BOTTOM
            # half (sinf[64:] = -sin) both muls read same-base-partition pairs
            # (a birverifier requirement for SB+SB tensor_tensor).
            nc.vector.tensor_mul(rot[0:64, :], raw[64:128, :], sn[64:128, :])
            nc.vector.tensor_mul(rot[64:128, :], raw[0:64, :], sn[0:64, :])
            nc.vector.tensor_add(dst[:], dst[:], rot[:])

        def q_drain(ps, j, f):
            qt = qkp.tile([128, 512], F16, tag=f"qt{f}", name=f"qt{j}{f}")
            qts[(j, f)] = qt
            rope_drain(ps, j, qt)

        def k_drain(ps, j):
            # K slices are read by every later B(j') — persistent, one per j.
            kt = vp.tile([128, 512], F16, tag=f"kt{j}", name=f"kt{j}")
            kts[j] = kt
            rope_drain(ps, j, kt)

        def v_drain(ps, j):
            v = vp.tile([128, 512], F16, tag=f"v{j}", name=f"v{j}")
            vsb[j] = v
            nc.scalar.copy(v[:], ps[:])

        # ---------------- A(0): k-outer so compute starts on chunk 0 --------
        with tc.tile_pool(name="psA0", bufs=1, space="PSUM") as psA0:
            qps = [psA0.tile([128, 512], F32, tag=f"p{f}", name=f"qps{f}")
                   for f in range(NHQ)]
            kps = psA0.tile([128, 512], F32, tag="p4", name="kps")
            vps = psA0.tile([128, 512], F32, tag="p5", name="vps")
            def a0_mms(f, kk):
                rhs = xa[(0, kk // 8)][:, kk % 8, :]
                st, sp = (kk == 0), (kk == KT - 1)
                if f < 4:
                    nc.tensor.matmul(qps[f][:], wq_sb[:, kk, 128 * f:128 * (f + 1)],
                                     rhs, start=st, stop=sp, skip_group_check=True)
                elif f == 4:
                    nc.tensor.matmul(kps[:], wk_sb[:, kk, :], rhs,
                                     start=st, stop=sp, skip_group_check=True)
                else:
                    for i in range(4):
                        # start clears the whole PSUM bank, so only the first
                        # sub-block group may assert it.
                        nc.tensor.matmul(vps[:, 128 * i:128 * (i + 1)],
                                         xa[(0, kk // 8)][:, kk % 8,
                                                          128 * i:128 * (i + 1)],
                                         wv_sb[:, kk, :],
                                         start=(st and i == 0), stop=sp,
                                         skip_group_check=True)

            for kk in range(KT - 12):
                for f in range(6):
                    a0_mms(f, kk)
            # staggered tails: finish+drain K, q0, V first so B(0) can start
            # as soon as the last tail lands (K's rope leads the DVE queue).
            for f in (4, 0, 5, 1, 2, 3):
                for kk in range(KT - 12, KT):
                    a0_mms(f, kk)
                if f < 4:
                    q_drain(qps[f], 0, f)
                elif f == 4:
                    k_drain(kps, 0)
                else:
                    v_drain(vps, 0)

        # Steady-state PSUM pools, created after psA0's 6 banks are released.
        psA = ctx.enter_context(tc.tile_pool(name="psA", bufs=1, space="PSUM"))
        psB = ctx.enter_context(tc.tile_pool(name="psB", bufs=3, space="PSUM"))
        psO = ctx.enter_context(tc.tile_pool(name="psO", bufs=2, space="PSUM"))
        psC = ctx.enter_context(tc.tile_pool(name="psC", bufs=2, space="PSUM"))

        # ---------------- quantum generators for filler ----------------------
        def a_quanta(j):
            """A(j) for j>=1, output-outer: 6 chains x 8 pieces of ~850ns."""
            out = deque()
            state = {}

            def piece_fn(f, piece):
                def run():
                    if piece == 0:
                        state[f] = psA.tile([128, 512], F32, tag="aps",
                                            name=f"aps{j}{f}")
                    ps = state[f]
                    if f < 5:
                        for kl in range(4):
                            kk = 4 * piece + kl
                            w = (wq_sb[:, kk, 128 * f:128 * (f + 1)] if f < 4
                                 else wk_sb[:, kk, :])
                            nc.tensor.matmul(ps[:], w, xa[(j, kk // 8)][:, kk % 8, :],
                                             start=(kk == 0), stop=(kk == KT - 1),
                                             skip_group_check=True)
                    else:
                        for m in range(16):
                            idx = 16 * piece + m
                            kk, i = divmod(idx, 4)
                            nc.tensor.matmul(ps[:, 128 * i:128 * (i + 1)],
                                             xa[(j, kk // 8)][:, kk % 8,
                                                              128 * i:128 * (i + 1)],
                                             wv_sb[:, kk, :],
                                             start=(kk == 0 and i == 0),
                                             stop=(kk == KT - 1),
                                             skip_group_check=True)
                    if piece == 7:
                        if f < 4:
                            q_drain(ps, j, f)
                        elif f == 4:
                            k_drain(ps, j)
                        else:
                            v_drain(ps, j)
                return run

            for f in range(6):
                for piece in range(8):
                    out.append(piece_fn(f, piece))
            return out

        def c_quanta(jc):
            """C(jc): 8 n-slices x 4 sq-subtiles, one ~850ns quantum each."""
            out = deque()
            state = {}

            def group_fn(n, q):
                def run():
                    if q == 0:
                        state[n] = ysp.tile([128, 4, 512], F16, tag="ys",
                                            name=f"ys{jc}{n}")
                    ys = state[n]
                    # during C(3) no A-chain or B work remains, so the idle
                    # psA and psO banks join the yp rotation (5 slots).
                    m = (4 * n + q) % 5
                    if jc == 3 and m == 2:
                        yp = psA.tile([128, 512], F32, tag="aps", name=f"yp{jc}{n}{q}")
                    elif jc == 3 and m == 4:
                        yp = psO.tile([128, 512], F32, tag="ops", name=f"yp{jc}{n}{q}")
                    else:
                        yp = psC.tile([128, 512], F32, tag="yp", name=f"yp{jc}{n}{q}")
                    for k in range(NHQ):
                        nc.tensor.matmul(yp[:], ots[(jc, k)][:, 128 * q:128 * (q + 1)],
                                         wo_sb[:, k, 512 * n:512 * (n + 1)],
                                         start=(k == 0), stop=(k == NHQ - 1),
                                         skip_group_check=True)
                    if (n + q) % 2 == 0:
                        nc.vector.tensor_copy(ys[:, q, :], yp[:])
                    else:
                        nc.scalar.copy(ys[:, q, :], yp[:])
                    # The very last group's writeback is split so the final DMA
                    # covers only one 128-row tile (shorter kernel tail).
                    if jc == 3 and n == NO - 1:
                        if q == 1:
                            nc.sync.dma_start(
                                out=y[512 * jc:512 * jc + 256, 512 * n:512 * (n + 1)]
                                .rearrange("(q p) m -> p q m", p=128),
                                in_=ys[:, 0:2, :])
                        elif q >= 2:
                            r0 = 512 * jc + 128 * q
                            nc.sync.dma_start(
                                out=y[r0:r0 + 128, 512 * n:512 * (n + 1)],
                                in_=ys[:, q, :])
                    elif q == 3:
                        nc.sync.dma_start(
                            out=y[512 * jc:512 * (jc + 1), 512 * n:512 * (n + 1)]
                            .rearrange("(q p) m -> p q m", p=128),
                            in_=ys[:])
                return run

            if jc == 3:
                # split-emit the first 5 groups: their k=0..2 contractions can
                # run during the last head's normalization-chain latency; the
                # k=3 matmuls (which wait on ot(3,3)) follow.
                ypre = {}

                def pre_fn(n, q):
                    def run():
                        if q == 0:
                            state[n] = ysp.tile([128, 4, 512], F16, tag="ys",
                                                name=f"ys{jc}{n}")
                        m = (4 * n + q) % 5
                        pool, tag = ((psA, "aps") if m == 2 else
                                     (psO, "ops") if m == 4 else (psC, "yp"))
                        yp = pool.tile([128, 512], F32, tag=tag,
                                       name=f"yp{jc}{n}{q}")
                        ypre[(n, q)] = yp
                        for k in range(NHQ - 1):
                            nc.tensor.matmul(
                                yp[:], ots[(jc, k)][:, 128 * q:128 * (q + 1)],
                                wo_sb[:, k, 512 * n:512 * (n + 1)],
                                start=(k == 0), stop=False,
                                skip_group_check=True)
                    return run

                def post_fn(n, q):
                    def run():
                        ys = state[n]
                        yp = ypre[(n, q)]
                        k = NHQ - 1
                        nc.tensor.matmul(
                            yp[:], ots[(jc, k)][:, 128 * q:128 * (q + 1)],
                            wo_sb[:, k, 512 * n:512 * (n + 1)],
                            start=False, stop=True, skip_group_check=True)
                        if (n + q) % 2 == 0:
                            nc.vector.tensor_copy(ys[:, q, :], yp[:])
                        else:
                            nc.scalar.copy(ys[:, q, :], yp[:])
                        if q == 3:
                            nc.sync.dma_start(
                                out=y[512 * jc:512 * (jc + 1),
                                      512 * n:512 * (n + 1)]
                                .rearrange("(q p) m -> p q m", p=128),
                                in_=ys[:])
                    return run

                for i in range(5):
                    out.append(pre_fn(i // 4, i % 4))
                for i in range(5):
                    out.append(post_fn(i // 4, i % 4))
                for i in range(5, NO * 4):
                    out.append(group_fn(i // 4, i % 4))
            else:
                for n in range(NO):
                    for q in range(4):
                        out.append(group_fn(n, q))
            return out

        fillerA = deque()
        fillerC = deque()

        def pull():
            if fillerC:
                fillerC.popleft()()
            elif fillerA:
                fillerA.popleft()()

        # ---------------- B(j): attention with interleaved filler ------------
        def emit_b(j):
            # Flat (h, t) loop: the pv pipeline (depth 3) carries across head
            # boundaries, so the exp-latency refill bubble is paid once per j
            # instead of once per head.
            tmax = 4 * j + 4
            state = {}
            pend = deque()
            cnt = 0

            def emit_pv(ent):
                h, t, ptb, c0 = ent
                ops, pacc = state[h]
                nc.tensor.matmul(ops[:, c0:512],
                                 vsb[t // 4][:, 128 * (t % 4):128 * (t % 4 + 1)],
                                 ptb[:, c0:512], start=(t == 0),
                                 stop=(t == tmax - 1), skip_group_check=True)
                if t == tmax - 1:
                    lsum = nrmp.tile([128, 512], F32, tag="lsum",
                                     name=f"lsum{j}{h}", bufs=1)
                    nc.gpsimd.partition_all_reduce(
                        lsum[:], pacc[:], channels=128,
                        reduce_op=bass_isa.ReduceOp.add)
                    rb = nrmp.tile([128, 512], F16, tag="rb", name=f"rb{j}{h}",
                                   bufs=1)
                    nc.vector.reciprocal(rb[:], lsum[:])
                    ot = otp.tile([128, 512], F16, tag=f"ot{h}", name=f"ot{j}{h}")
                    ots[(j, h)] = ot
                    nc.vector.tensor_mul(ot[:], ops[:], rb[:])

            for h in range(NHQ):
                for t in range(tmax):
                    if t == 0:
                        state[h] = (
                            psO.tile([128, 512], F32, tag="ops", name=f"ops{j}{h}"),
                            paccp.tile([128, 512], F16, tag="pacc",
                                       name=f"pacc{j}{h}"))
                    ops, pacc = state[h]
                    r = t - 4 * j
                    c0 = 128 * r if r >= 0 else 0
                    scp = psB.tile([128, 512], F32, tag="scp", name=f"scp{j}{h}{t}")
                    nc.tensor.matmul(scp[:, c0:512],
                                     kts[t // 4][:, 128 * (t % 4):128 * (t % 4 + 1)],
                                     qts[(j, h)][:, c0:512], start=True, stop=True,
                                     skip_group_check=True)
                    if r >= 0:
                        nc.vector.tensor_add(scp[:, c0:c0 + 128],
                                             scp[:, c0:c0 + 128], mask_sb[:])
                    ptb = ptbp.tile([128, 512], F16, tag="ptb", name=f"ptb{j}{h}{t}")
                    nc.scalar.activation(ptb[:, c0:512], scp[:, c0:512], EXP,
                                         bias=0.0, scale=SCALE)
                    if t == 0:
                        nc.vector.tensor_copy(pacc[:], ptb[:])
                    else:
                        nc.vector.tensor_add(pacc[:, c0:512], pacc[:, c0:512],
                                             ptb[:, c0:512])
                    pend.append((h, t, ptb, c0))
                    if len(pend) > 8:
                        emit_pv(pend.popleft())
                    cnt += 1
                    if j <= 2:
                        pull()
                        pull()
                    elif cnt % 2 == 0:
                        pull()
            while pend:
                emit_pv(pend.popleft())

        # ---------------- main fused j loop ----------------------------------
        for j in range(SL):
            while fillerA:
                fillerA.popleft()()              # A(j) leftover
            if j + 2 < SL:
                issue_x(j + 2)
            if j + 1 < SL:
                fillerA.extend(a_quanta(j + 1))
            emit_b(j)
            while fillerC:
                fillerC.popleft()()              # C(j-1) leftover
            fillerC.extend(c_quanta(j))
        while fillerA:
            fillerA.popleft()()
        while fillerC:
            fillerC.popleft()()                  # C(3)

    nc.compile()
    return nc


def get_nc():
    if "nc" not in _NC_CACHE:
        _NC_CACHE["nc"] = build_nc()
    return _NC_CACHE["nc"]


def make_in_maps(hidden_states, attention_mask, position_ids, Wq, Wk, Wv, Wo):
    hs = np.asarray(hidden_states, dtype=np.float32)
    pos = np.asarray(position_ids)
    Wq = np.asarray(Wq, dtype=np.float32)
    Wk = np.asarray(Wk, dtype=np.float32)
    Wv = np.asarray(Wv, dtype=np.float32)
    Wo = np.asarray(Wo, dtype=np.float32)
    assert hs.shape == (1, S, HID)
    assert Wq.shape == (HID, HID) and Wk.shape == (HID, 1024)
    assert Wv.shape == (HID, 1024) and Wo.shape == (HID, HID)

    # x_r[p, t, s] = hidden[0, s, 128t+p]
    x_r = np.ascontiguousarray(
        hs[0].T.reshape(KT, 128, S).transpose(1, 0, 2)).astype(np.float16)

    p = pos[0].astype(np.float32)
    inv = (1.0 / (10000.0 ** (np.arange(0, D, 2, dtype=np.float32)
                              / np.float32(D)))).astype(np.float32)
    freqs = p[:, None] * inv[None, :]
    emb = np.concatenate([freqs, freqs], axis=1)        # (S, 128)
    cos_r = np.ascontiguousarray(np.cos(emb).T).astype(np.float16)
    sinT = np.sin(emb).T.astype(np.float32)
    sinT[64:] *= np.float32(-1.0)
    sinf_r = np.ascontiguousarray(sinT).astype(np.float16)

    ar = np.arange(128)
    mask_r = np.where(ar[:, None] <= ar[None, :], np.float32(0),
                      np.float32(-1e9 / SCALE)).astype(np.float32)

    in_maps = []
    for c in range(NCORES):
        wq_c = Wq[:, 512 * c:512 * (c + 1)]
        wk_c = Wk[:, 128 * c:128 * (c + 1)]
        wv_c = Wv[:, 128 * c:128 * (c + 1)]
        wo_c = Wo[512 * c:512 * (c + 1), :]
        w_pack = np.concatenate([wq_c, wk_c, wv_c], axis=1)     # [HID, 768]
        in_maps.append({
            "x_r": x_r,
            "w_r": np.ascontiguousarray(
                w_pack.reshape(KT, 128, 768).transpose(1, 0, 2)).astype(np.float16),
            "wo_r": np.ascontiguousarray(
                wo_c.reshape(NHQ, 128, HID).transpose(1, 0, 2)).astype(np.float16),
            "cos_r": cos_r,
            "sinf_r": sinf_r,
            "mask_r": mask_r,
        })
    return in_maps


def kernel(hidden_states, attention_mask, position_ids, Wq, Wk, Wv, Wo):
    # The axon NTFF trace hook isn't shipped in this container; make sure a
    # stray BASS_TRACE in the environment can't route us onto that path.
    os.environ["BASS_NEVER_TRACE"] = "1"
    in_maps = make_in_maps(hidden_states, attention_mask, position_ids,
                           Wq, Wk, Wv, Wo)
    nc = get_nc()
    res = run_bass_kernel_spmd(nc, in_maps, list(range(NCORES)))
    acc = np.zeros((S, HID), dtype=np.float64)
    for c in range(NCORES):
        acc += res.results[c]["y"]
    return acc.astype(np.float32)[None]


# revision 98
# speedup vs baseline: 1.0005x; 1.0005x over previous
"""Grouped-query attention (B=1, S=2048, HID=4096, 32 q-heads / 8 kv-heads,
D=128, RoPE, additive causal mask) on 8 Trainium2 NeuronCores.

Sharding: tensor-parallel over heads. Core c owns 4 q-heads (columns
512c:512c+512 of Wq), kv-head c (columns 128c:128c+128 of Wk/Wv), and rows
512c:512c+512 of Wo. Each core emits a full-shape fp16 partial of the output
projection; the host sums the 8 partials (the all-reduce of the row-sharded
Wo matmul).

v2 design — all-fp16 data path, fused j-slice pipeline:
  - Everything on the PE runs in fp16 (1 cycle/row at any moving size;
    fp32 accumulation in PSUM), halving all DMA traffic vs fp32.
  - Work is organized as a single pipeline over the four 512-query slices j:
    A(j) projections+RoPE -> B(j) attention -> C(j) output projection.
    B(j)'s inner loop is rate-limited by the scalar engine's exp, so A(j+1)
    and C(j-1) tensor-engine work is interleaved into B(j)'s matmul stream
    as ~850ns filler quanta, keeping the PE gapless (each PE idle gap costs
    ~600ns of p-state penalty on top of the stall).
  - Scores are computed transposed (sc[sk,sq]) so exp output feeds the PV
    matmul directly. The softmax denominator is accumulated on the vector
    engine (Pacc += P_t), reduced+broadcast across partitions by the idle
    GPSIMD engine (partition_all_reduce), avoiding ones-matmuls on the PE.
  - V is projected untransposed (lhsT = x^T tile, rhs = Wv tile) so no
    PE/vector transposes are needed anywhere.
  - Causal structure: off-diagonal score tiles are skipped entirely; the
    four diagonal tiles per (h,j) are computed width-restricted
    (512-128r columns) with a single constant 128x128 additive mask.
  - Host pre-rearranges all inputs partition-major so every DMA is a
    straight wide-descriptor copy.
"""
import os
from collections import deque
from contextlib import ExitStack

import numpy as np

import concourse.bass as bass
import concourse.tile as tile
from concourse import bacc, bass_isa, mybir
from concourse.bass_utils import run_bass_kernel_spmd

F32 = mybir.dt.float32
F16 = mybir.dt.float16
EXP = mybir.ActivationFunctionType.Exp

S = 2048
HID = 4096
D = 128
NCORES = 8
NHQ = 4                      # q heads per core
SCALE = float(D) ** -0.5
SL = 4                       # 512-query j slices
KT = HID // 128              # 32 hidden contraction tiles
NO = HID // 512              # 8 output column slices

_NC_CACHE = {}


def build_nc():
    nc = bacc.Bacc("TRN2", target_bir_lowering=False, debug=False,
                   num_devices=NCORES)
    x_r = nc.dram_tensor("x_r", [128, KT, S], F16, kind="ExternalInput").ap()
    # packed projection weights: cols 0:512 = Wq, 512:640 = Wk, 640:768 = Wv
    w_r = nc.dram_tensor("w_r", [128, KT, 768], F16, kind="ExternalInput").ap()
    wo_r = nc.dram_tensor("wo_r", [128, NHQ, HID], F16, kind="ExternalInput").ap()
    cos_r = nc.dram_tensor("cos_r", [128, S], F16, kind="ExternalInput").ap()
    sinf_r = nc.dram_tensor("sinf_r", [128, S], F16, kind="ExternalInput").ap()
    mask_r = nc.dram_tensor("mask_r", [128, 128], F32, kind="ExternalInput").ap()
    y = nc.dram_tensor("y", [S, HID], F16, kind="ExternalOutput").ap()

    with tile.TileContext(nc) as tc, ExitStack() as ctx, \
            nc.allow_low_precision(reason="fp16 data path validated vs 2e-2 gate"):
        const = ctx.enter_context(tc.tile_pool(name="const", bufs=1))
        xap = ctx.enter_context(tc.tile_pool(name="xap", bufs=8))
        qkp = ctx.enter_context(tc.tile_pool(name="qkp", bufs=2))
        vp = ctx.enter_context(tc.tile_pool(name="vp", bufs=1))
        otp = ctx.enter_context(tc.tile_pool(name="otp", bufs=2))
        tmp = ctx.enter_context(tc.tile_pool(name="tmp", bufs=2))
        ptbp = ctx.enter_context(tc.tile_pool(name="ptbp", bufs=10))
        paccp = ctx.enter_context(tc.tile_pool(name="paccp", bufs=2))
        nrmp = ctx.enter_context(tc.tile_pool(name="nrmp", bufs=2))
        ysp = ctx.enter_context(tc.tile_pool(name="ysp", bufs=2))

        w_sb = const.tile([128, KT, 768], F16)
        wq_sb = w_sb[:, :, 0:512]
        wk_sb = w_sb[:, :, 512:640]
        wv_sb = w_sb[:, :, 640:768]
        wo_sb = const.tile([128, NHQ, HID], F16)
        cos_sb = const.tile([128, S], F16)
        sinf_sb = const.tile([128, S], F16)
        mask_sb = const.tile([128, 128], F32)

        xa = {}      # (j, g) -> [128, 8, 512] x^T quarter-slice
        qts = {}     # (j, h) -> [128, 512] roped Q^T slice
        kts = {}     # j -> [128, 512] roped K^T slice
        vsb = {}     # j -> [128, 512] V slice ([s-in-block, d] layout)
        ots = {}     # (j, h) -> [128, 512] normalized attention out^T

        def issue_x(j):
            for g in range(4):
                t = xap.tile([128, 8, 512], F16, tag="xa", name=f"xa{j}{g}")
                xa[(j, g)] = t
                nc.sync.dma_start(out=t[:], in_=x_r[:, 8 * g:8 * g + 8,
                                                    512 * j:512 * (j + 1)])

        # DMA priority order: per-g weight chunks interleaved with x(0) so the
        # k-outer A(0) can start after the first chunk group; then the rest.
        # The first group is split in half so compute starts ~3.5us sooner.
        for g in range(4):
            subs = ([slice(0, 1), slice(1, 2), slice(2, 4), slice(4, 6),
                     slice(6, 8)] if g == 0
                    else [slice(8 * g, 8 * g + 4), slice(8 * g + 4, 8 * g + 8)])
            t = xap.tile([128, 8, 512], F16, tag="xa", name=f"xa0{g}")
            xa[(0, g)] = t
            for gs in subs:
                nc.sync.dma_start(out=w_sb[:, gs, :], in_=w_r[:, gs, :])
                ls = slice(gs.start - 8 * g, gs.stop - 8 * g)
                nc.sync.dma_start(out=t[:, ls, :], in_=x_r[:, gs, 0:512])
            if g == 1:
                # mid-stream so they land before A(0)'s rope drains, without
                # delaying either the first chunks or x(1).
                nc.sync.dma_start(out=cos_sb[:], in_=cos_r[:])
                nc.sync.dma_start(out=sinf_sb[:], in_=sinf_r[:])
        issue_x(1)
        nc.sync.dma_start(out=mask_sb[:], in_=mask_r[:])
        nc.sync.dma_start(out=wo_sb[:], in_=wo_r[:])

        def rope_drain(ps, j, dst):
            """dst = ps*cos + rotate_half(ps)*sin for slice j (sin sign-folded)."""
            raw = tmp.tile([128, 512], F16, tag="raw", name="raw", bufs=5)
            nc.scalar.copy(raw[:], ps[:])
            cs = cos_sb[:, 512 * j:512 * (j + 1)]
            sn = sinf_sb[:, 512 * j:512 * (j + 1)]
            rot = tmp.tile([128, 512], F16, tag="rot", name="rot", bufs=3)
            nc.vector.tensor_mul(dst[:], raw[:], cs)
            # sin's halves are identical, so with the sign fold on the BOTTOM
            # half (sinf[64:] = -sin) both muls read same-base-partition pairs
            # (a birverifier requirement for SB+SB tensor_tensor).
            nc.vector.tensor_mul(rot[0:64, :], raw[64:128, :], sn[64:128, :])
            nc.vector.tensor_mul(rot[64:128, :], raw[0:64, :], sn[0:64, :])
            nc.vector.tensor_add(dst[:], dst[:], rot[:])

        def q_drain(ps, j, f):
            qt = qkp.tile([128, 512], F16, tag=f"qt{f}", name=f"qt{j}{f}")
            qts[(j, f)] = qt
            rope_drain(ps, j, qt)

        def k_drain(ps, j):
            # K slices are read by every later B(j') — persistent, one per j.
            kt = vp.tile([128, 512], F16, tag=f"kt{j}", name=f"kt{j}")
            kts[j] = kt
            rope_drain(ps, j, kt)

        def v_drain(ps, j):
            v = vp.tile([128, 512], F16, tag=f"v{j}", name=f"v{j}")
            vsb[j] = v
            nc.scalar.copy(v[:], ps[:])

        # ---------------- A(0): k-outer so compute starts on chunk 0 --------
        with tc.tile_pool(name="psA0", bufs=1, space="PSUM") as psA0:
            qps = [psA0.tile([128, 512], F32, tag=f"p{f}", name=f"qps{f}")
                   for f in range(NHQ)]
            kps = psA0.tile([128, 512], F32, tag="p4", name="kps")
            vps = psA0.tile([128, 512], F32, tag="p5", name="vps")
            def a0_mms(f, kk):
                rhs = xa[(0, kk // 8)][:, kk % 8, :]
                st, sp = (kk == 0), (kk == KT - 1)
                if f < 4:
                    nc.tensor.matmul(qps[f][:], wq_sb[:, kk, 128 * f:128 * (f + 1)],
                                     rhs, start=st, stop=sp, skip_group_check=True)
                elif f == 4:
                    nc.tensor.matmul(kps[:], wk_sb[:, kk, :], rhs,
                                     start=st, stop=sp, skip_group_check=True)
                else:
                    for i in range(4):
                        # start clears the whole PSUM bank, so only the first
                        # sub-block group may assert it.
                        nc.tensor.matmul(vps[:, 128 * i:128 * (i + 1)],
                                         xa[(0, kk // 8)][:, kk % 8,
                                                          128 * i:128 * (i + 1)],
                                         wv_sb[:, kk, :],
                                         start=(st and i == 0), stop=sp,
                                         skip_group_check=True)

            for kk in range(KT - 12):
                for f in range(6):
                    a0_mms(f, kk)
            # staggered tails: finish+drain K, q0, V first so B(0) can start
            # as soon as the last tail lands (K's rope leads the DVE queue).
            for f in (4, 0, 5, 1, 2, 3):
                for kk in range(KT - 12, KT):
                    a0_mms(f, kk)
                if f < 4:
                    q_drain(qps[f], 0, f)
                elif f == 4:
                    k_drain(kps, 0)
                else:
                    v_drain(vps, 0)

        # Steady-state PSUM pools, created after psA0's 6 banks are released.
        psA = ctx.enter_context(tc.tile_pool(name="psA", bufs=1, space="PSUM"))
        psB = ctx.enter_context(tc.tile_pool(name="psB", bufs=3, space="PSUM"))
        psO = ctx.enter_context(tc.tile_pool(name="psO", bufs=2, space="PSUM"))
        psC = ctx.enter_context(tc.tile_pool(name="psC", bufs=2, space="PSUM"))

        # ---------------- quantum generators for filler ----------------------
        def a_quanta(j):
            """A(j) for j>=1, output-outer: 6 chains x 8 pieces of ~850ns."""
            out = deque()
            state = {}

            def piece_fn(f, piece):
                def run():
                    if piece == 0:
                        state[f] = psA.tile([128, 512], F32, tag="aps",
                                            name=f"aps{j}{f}")
                    ps = state[f]
                    if f < 5:
                        for kl in range(4):
                            kk = 4 * piece + kl
                            w = (wq_sb[:, kk, 128 * f:128 * (f + 1)] if f < 4
                                 else wk_sb[:, kk, :])
                            nc.tensor.matmul(ps[:], w, xa[(j, kk // 8)][:, kk % 8, :],
                                             start=(kk == 0), stop=(kk == KT - 1),
                                             skip_group_check=True)
                    else:
                        for m in range(16):
                            idx = 16 * piece + m
                            kk, i = divmod(idx, 4)
                            nc.tensor.matmul(ps[:, 128 * i:128 * (i + 1)],
                                             xa[(j, kk // 8)][:, kk % 8,
                                                              128 * i:128 * (i + 1)],
                                             wv_sb[:, kk, :],
                                             start=(kk == 0 and i == 0),
                                             stop=(kk == KT - 1),
                                             skip_group_check=True)
                    if piece == 7:
                        if f < 4:
                            q_drain(ps, j, f)
                        elif f == 4:
                            k_drain(ps, j)
                        else:
                            v_drain(ps, j)
                return run

            for f in range(6):
                for piece in range(8):
                    out.append(piece_fn(f, piece))
            return out

        def c_quanta(jc):
            """C(jc): 8 n-slices x 4 sq-subtiles, one ~850ns quantum each."""
            out = deque()
            state = {}

            def group_fn(n, q):
                def run():
                    if q == 0:
                        state[n] = ysp.tile([128, 4, 512], F16, tag="ys",
                                            name=f"ys{jc}{n}")
                    ys = state[n]
                    # during C(3) no A-chain or B work remains, so the idle
                    # psA and psO banks join the yp rotation (5 slots).
                    m = (4 * n + q) % 5
                    if jc == 3 and m == 2:
                        yp = psA.tile([128, 512], F32, tag="aps", name=f"yp{jc}{n}{q}")
                    elif jc == 3 and m == 4:
                        yp = psO.tile([128, 512], F32, tag="ops", name=f"yp{jc}{n}{q}")
                    else:
                        yp = psC.tile([128, 512], F32, tag="yp", name=f"yp{jc}{n}{q}")
                    for k in range(NHQ):
                        nc.tensor.matmul(yp[:], ots[(jc, k)][:, 128 * q:128 * (q + 1)],
                                         wo_sb[:, k, 512 * n:512 * (n + 1)],
                                         start=(k == 0), stop=(k == NHQ - 1),
                                         skip_group_check=True)
                    if (n + q) % 2 == 0:
                        nc.vector.tensor_copy(ys[:, q, :], yp[:])
                    else:
                        nc.scalar.copy(ys[:, q, :], yp[:])
                    # The very last group's writeback is split so the final DMA
                    # covers only one 128-row tile (shorter kernel tail).
                    if jc == 3 and n == NO - 1:
                        if q == 1:
                            nc.sync.dma_start(
                                out=y[512 * jc:512 * jc + 256, 512 * n:512 * (n + 1)]
                                .rearrange("(q p) m -> p q m", p=128),
                                in_=ys[:, 0:2, :])
                        elif q >= 2:
                            r0 = 512 * jc + 128 * q
                            nc.sync.dma_start(
                                out=y[r0:r0 + 128, 512 * n:512 * (n + 1)],
                                in_=ys[:, q, :])
                    elif q == 3:
                        nc.sync.dma_start(
                            out=y[512 * jc:512 * (jc + 1), 512 * n:512 * (n + 1)]
                            .rearrange("(q p) m -> p q m", p=128),
                            in_=ys[:])
                return run

            if jc == 3:
                # split-emit the first 5 groups: their k=0..2 contractions can
                # run during the last head's normalization-chain latency; the
                # k=3 matmuls (which wait on ot(3,3)) follow.
                ypre = {}

                def pre_fn(n, q, i):
                    def run():
                        if q == 0:
                            state[n] = ysp.tile([128, 4, 512], F16, tag="ys",
                                                name=f"ys{jc}{n}")
                        # five DISTINCT banks so no pre-group waits on another
                        pool, tag = [(psC, "yp"), (psC, "yp"), (psA, "aps"),
                                     (psO, "ops"), (psO, "ops")][i]
                        yp = pool.tile([128, 512], F32, tag=tag,
                                       name=f"yp{jc}{n}{q}")
                        ypre[(n, q)] = yp
                        for k in range(NHQ - 1):
                            nc.tensor.matmul(
                                yp[:], ots[(jc, k)][:, 128 * q:128 * (q + 1)],
                                wo_sb[:, k, 512 * n:512 * (n + 1)],
                                start=(k == 0), stop=False,
                                skip_group_check=True)
                    return run

                def post_fn(n, q):
                    def run():
                        ys = state[n]
                        yp = ypre[(n, q)]
                        k = NHQ - 1
                        nc.tensor.matmul(
                            yp[:], ots[(jc, k)][:, 128 * q:128 * (q + 1)],
                            wo_sb[:, k, 512 * n:512 * (n + 1)],
                            start=False, stop=True, skip_group_check=True)
                        if (n + q) % 2 == 0:
                            nc.vector.tensor_copy(ys[:, q, :], yp[:])
                        else:
                            nc.scalar.copy(ys[:, q, :], yp[:])
                        if q == 3:
                            nc.sync.dma_start(
                                out=y[512 * jc:512 * (jc + 1),
                                      512 * n:512 * (n + 1)]
                                .rearrange("(q p) m -> p q m", p=128),
                                in_=ys[:])
                    return run

                for i in range(5):
                    out.append(pre_fn(i // 4, i % 4, i))
                for i in range(5):
                    out.append(post_fn(i // 4, i % 4))
                for i in range(5, NO * 4):
                    out.append(group_fn(i // 4, i % 4))
            else:
                for n in range(NO):
                    for q in range(4):
                        out.append(group_fn(n, q))
            return out

        fillerA = deque()
        fillerC = deque()

        def pull():
            if fillerC:
                fillerC.popleft()()
            elif fillerA:
                fillerA.popleft()()

        # ---------------- B(j): attention with interleaved filler ------------
        def emit_b(j):
            # Flat (h, t) loop: the pv pipeline (depth 3) carries across head
            # boundaries, so the exp-latency refill bubble is paid once per j
            # instead of once per head.
            tmax = 4 * j + 4
            state = {}
            pend = deque()
            cnt = 0

            def emit_pv(ent):
                h, t, ptb, c0 = ent
                ops, pacc = state[h]
                nc.tensor.matmul(ops[:, c0:512],
                                 vsb[t // 4][:, 128 * (t % 4):128 * (t % 4 + 1)],
                                 ptb[:, c0:512], start=(t == 0),
                                 stop=(t == tmax - 1), skip_group_check=True)
                if t == tmax - 1:
                    lsum = nrmp.tile([128, 512], F32, tag="lsum",
                                     name=f"lsum{j}{h}", bufs=1)
                    nc.gpsimd.partition_all_reduce(
                        lsum[:], pacc[:], channels=128,
                        reduce_op=bass_isa.ReduceOp.add)
                    rb = nrmp.tile([128, 512], F16, tag="rb", name=f"rb{j}{h}",
                                   bufs=1)
                    nc.vector.reciprocal(rb[:], lsum[:])
                    ot = otp.tile([128, 512], F16, tag=f"ot{h}", name=f"ot{j}{h}")
                    ots[(j, h)] = ot
                    nc.vector.tensor_mul(ot[:], ops[:], rb[:])

            for h in range(NHQ):
                for t in range(tmax):
                    if t == 0:
                        state[h] = (
                            psO.tile([128, 512], F32, tag="ops", name=f"ops{j}{h}"),
                            paccp.tile([128, 512], F16, tag="pacc",
                                       name=f"pacc{j}{h}"))
                    ops, pacc = state[h]
                    r = t - 4 * j
                    c0 = 128 * r if r >= 0 else 0
                    scp = psB.tile([128, 512], F32, tag="scp", name=f"scp{j}{h}{t}")
                    nc.tensor.matmul(scp[:, c0:512],
                                     kts[t // 4][:, 128 * (t % 4):128 * (t % 4 + 1)],
                                     qts[(j, h)][:, c0:512], start=True, stop=True,
                                     skip_group_check=True)
                    if r >= 0:
                        nc.vector.tensor_add(scp[:, c0:c0 + 128],
                                             scp[:, c0:c0 + 128], mask_sb[:])
                    ptb = ptbp.tile([128, 512], F16, tag="ptb", name=f"ptb{j}{h}{t}")
                    nc.scalar.activation(ptb[:, c0:512], scp[:, c0:512], EXP,
                                         bias=0.0, scale=SCALE)
                    if t == 0:
                        nc.vector.tensor_copy(pacc[:], ptb[:])
                    else:
                        nc.vector.tensor_add(pacc[:, c0:512], pacc[:, c0:512],
                                             ptb[:, c0:512])
                    pend.append((h, t, ptb, c0))
                    if len(pend) > 8:
                        emit_pv(pend.popleft())
                    cnt += 1
                    if j <= 2:
                        pull()
                        pull()
                    elif cnt % 2 == 0:
                        pull()
            while pend:
                emit_pv(pend.popleft())

        # ---------------- main fused j loop ----------------------------------
        for j in range(SL):
            while fillerA:
                fillerA.popleft()()              # A(j) leftover
            if j + 2 < SL:
                issue_x(j + 2)
            if j + 1 < SL:
                fillerA.extend(a_quanta(j + 1))
            emit_b(j)
            while fillerC:
                fillerC.popleft()()              # C(j-1) leftover
            fillerC.extend(c_quanta(j))
        while fillerA:
            fillerA.popleft()()
        while fillerC:
            fillerC.popleft()()                  # C(3)

    nc.compile()
    return nc


def get_nc():
    if "nc" not in _NC_CACHE:
        _NC_CACHE["nc"] = build_nc()
    return _NC_CACHE["nc"]


def make_in_maps(hidden_states, attention_mask, position_ids, Wq, Wk, Wv, Wo):
    hs = np.asarray(hidden_states, dtype=np.float32)
    pos = np.asarray(position_ids)
    Wq = np.asarray(Wq, dtype=np.float32)
    Wk = np.asarray(Wk, dtype=np.float32)
    Wv = np.asarray(Wv, dtype=np.float32)
    Wo = np.asarray(Wo, dtype=np.float32)
    assert hs.shape == (1, S, HID)
    assert Wq.shape == (HID, HID) and Wk.shape == (HID, 1024)
    assert Wv.shape == (HID, 1024) and Wo.shape == (HID, HID)

    # x_r[p, t, s] = hidden[0, s, 128t+p]
    x_r = np.ascontiguousarray(
        hs[0].T.reshape(KT, 128, S).transpose(1, 0, 2)).astype(np.float16)

    p = pos[0].astype(np.float32)
    inv = (1.0 / (10000.0 ** (np.arange(0, D, 2, dtype=np.float32)
                              / np.float32(D)))).astype(np.float32)
    freqs = p[:, None] * inv[None, :]
    emb = np.concatenate([freqs, freqs], axis=1)        # (S, 128)
    cos_r = np.ascontiguousarray(np.cos(emb).T).astype(np.float16)
    sinT = np.sin(emb).T.astype(np.float32)
    sinT[64:] *= np.float32(-1.0)
    sinf_r = np.ascontiguousarray(sinT).astype(np.float16)

    ar = np.arange(128)
    mask_r = np.where(ar[:, None] <= ar[None, :], np.float32(0),
                      np.float32(-1e9 / SCALE)).astype(np.float32)

    in_maps = []
    for c in range(NCORES):
        wq_c = Wq[:, 512 * c:512 * (c + 1)]
        wk_c = Wk[:, 128 * c:128 * (c + 1)]
        wv_c = Wv[:, 128 * c:128 * (c + 1)]
        wo_c = Wo[512 * c:512 * (c + 1), :]
        w_pack = np.concatenate([wq_c, wk_c, wv_c], axis=1)     # [HID, 768]
        in_maps.append({
            "x_r": x_r,
            "w_r": np.ascontiguousarray(
                w_pack.reshape(KT, 128, 768).transpose(1, 0, 2)).astype(np.float16),
            "wo_r": np.ascontiguousarray(
                wo_c.reshape(NHQ, 128, HID).transpose(1, 0, 2)).astype(np.float16),
            "cos_r": cos_r,
            "sinf_r": sinf_r,
            "mask_r": mask_r,
        })
    return in_maps


def kernel(hidden_states, attention_mask, position_ids, Wq, Wk, Wv, Wo):
    # The axon NTFF trace hook isn't shipped in this container; make sure a
    # stray BASS_TRACE in the environment can't route us onto that path.
    os.environ["BASS_NEVER_TRACE"] = "1"
    in_maps = make_in_maps(hidden_states, attention_mask, position_ids,
                           Wq, Wk, Wv, Wo)
    nc = get_nc()
    res = run_bass_kernel_spmd(nc, in_maps, list(range(NCORES)))
    acc = np.zeros((S, HID), dtype=np.float64)
    for c in range(NCORES):
        acc += res.results[c]["y"]
    return acc.astype(np.float32)[None]


# revision 99
# speedup vs baseline: 1.0015x; 1.0010x over previous
"""Grouped-query attention (B=1, S=2048, HID=4096, 32 q-heads / 8 kv-heads,
D=128, RoPE, additive causal mask) on 8 Trainium2 NeuronCores.

Sharding: tensor-parallel over heads. Core c owns 4 q-heads (columns
512c:512c+512 of Wq), kv-head c (columns 128c:128c+128 of Wk/Wv), and rows
512c:512c+512 of Wo. Each core emits a full-shape fp16 partial of the output
projection; the host sums the 8 partials (the all-reduce of the row-sharded
Wo matmul).

v2 design — all-fp16 data path, fused j-slice pipeline:
  - Everything on the PE runs in fp16 (1 cycle/row at any moving size;
    fp32 accumulation in PSUM), halving all DMA traffic vs fp32.
  - Work is organized as a single pipeline over the four 512-query slices j:
    A(j) projections+RoPE -> B(j) attention -> C(j) output projection.
    B(j)'s inner loop is rate-limited by the scalar engine's exp, so A(j+1)
    and C(j-1) tensor-engine work is interleaved into B(j)'s matmul stream
    as ~850ns filler quanta, keeping the PE gapless (each PE idle gap costs
    ~600ns of p-state penalty on top of the stall).
  - Scores are computed transposed (sc[sk,sq]) so exp output feeds the PV
    matmul directly. The softmax denominator is accumulated on the vector
    engine (Pacc += P_t), reduced+broadcast across partitions by the idle
    GPSIMD engine (partition_all_reduce), avoiding ones-matmuls on the PE.
  - V is projected untransposed (lhsT = x^T tile, rhs = Wv tile) so no
    PE/vector transposes are needed anywhere.
  - Causal structure: off-diagonal score tiles are skipped entirely; the
    four diagonal tiles per (h,j) are computed width-restricted
    (512-128r columns) with a single constant 128x128 additive mask.
  - Host pre-rearranges all inputs partition-major so every DMA is a
    straight wide-descriptor copy.
"""
import os
from collections import deque
from contextlib import ExitStack

import numpy as np

import concourse.bass as bass
import concourse.tile as tile
from concourse import bacc, bass_isa, mybir
from concourse.bass_utils import run_bass_kernel_spmd

F32 = mybir.dt.float32
F16 = mybir.dt.float16
EXP = mybir.ActivationFunctionType.Exp

S = 2048
HID = 4096
D = 128
NCORES = 8
NHQ = 4                      # q heads per core
SCALE = float(D) ** -0.5
SL = 4                       # 512-query j slices
KT = HID // 128              # 32 hidden contraction tiles
NO = HID // 512              # 8 output column slices

_NC_CACHE = {}


def build_nc():
    nc = bacc.Bacc("TRN2", target_bir_lowering=False, debug=False,
                   num_devices=NCORES)
    x_r = nc.dram_tensor("x_r", [128, KT, S], F16, kind="ExternalInput").ap()
    # packed projection weights: cols 0:512 = Wq, 512:640 = Wk, 640:768 = Wv
    w_r = nc.dram_tensor("w_r", [128, KT, 768], F16, kind="ExternalInput").ap()
    wo_r = nc.dram_tensor("wo_r", [128, NHQ, HID], F16, kind="ExternalInput").ap()
    cos_r = nc.dram_tensor("cos_r", [128, S], F16, kind="ExternalInput").ap()
    sinf_r = nc.dram_tensor("sinf_r", [128, S], F16, kind="ExternalInput").ap()
    mask_r = nc.dram_tensor("mask_r", [128, 128], F32, kind="ExternalInput").ap()
    y = nc.dram_tensor("y", [S, HID], F16, kind="ExternalOutput").ap()

    with tile.TileContext(nc) as tc, ExitStack() as ctx, \
            nc.allow_low_precision(reason="fp16 data path validated vs 2e-2 gate"):
        const = ctx.enter_context(tc.tile_pool(name="const", bufs=1))
        xap = ctx.enter_context(tc.tile_pool(name="xap", bufs=8))
        qkp = ctx.enter_context(tc.tile_pool(name="qkp", bufs=2))
        vp = ctx.enter_context(tc.tile_pool(name="vp", bufs=1))
        otp = ctx.enter_context(tc.tile_pool(name="otp", bufs=2))
        tmp = ctx.enter_context(tc.tile_pool(name="tmp", bufs=2))
        ptbp = ctx.enter_context(tc.tile_pool(name="ptbp", bufs=10))
        paccp = ctx.enter_context(tc.tile_pool(name="paccp", bufs=2))
        nrmp = ctx.enter_context(tc.tile_pool(name="nrmp", bufs=2))
        ysp = ctx.enter_context(tc.tile_pool(name="ysp", bufs=2))

        w_sb = const.tile([128, KT, 768], F16)
        wq_sb = w_sb[:, :, 0:512]
        wk_sb = w_sb[:, :, 512:640]
        wv_sb = w_sb[:, :, 640:768]
        wo_sb = const.tile([128, NHQ, HID], F16)
        cos_sb = const.tile([128, S], F16)
        sinf_sb = const.tile([128, S], F16)
        mask_sb = const.tile([128, 128], F32)

        xa = {}      # (j, g) -> [128, 8, 512] x^T quarter-slice
        qts = {}     # (j, h) -> [128, 512] roped Q^T slice
        kts = {}     # j -> [128, 512] roped K^T slice
        vsb = {}     # j -> [128, 512] V slice ([s-in-block, d] layout)
        ots = {}     # (j, h) -> [128, 512] normalized attention out^T

        def issue_x(j):
            for g in range(4):
                t = xap.tile([128, 8, 512], F16, tag="xa", name=f"xa{j}{g}")
                xa[(j, g)] = t
                nc.sync.dma_start(out=t[:], in_=x_r[:, 8 * g:8 * g + 8,
                                                    512 * j:512 * (j + 1)])

        # DMA priority order: per-g weight chunks interleaved with x(0) so the
        # k-outer A(0) can start after the first chunk group; then the rest.
        # The first group is split in half so compute starts ~3.5us sooner.
        for g in range(4):
            subs = ([slice(0, 1), slice(1, 2), slice(2, 4), slice(4, 6),
                     slice(6, 8)] if g == 0
                    else [slice(8 * g, 8 * g + 4), slice(8 * g + 4, 8 * g + 8)])
            t = xap.tile([128, 8, 512], F16, tag="xa", name=f"xa0{g}")
            xa[(0, g)] = t
            for gs in subs:
                nc.sync.dma_start(out=w_sb[:, gs, :], in_=w_r[:, gs, :])
                ls = slice(gs.start - 8 * g, gs.stop - 8 * g)
                nc.sync.dma_start(out=t[:, ls, :], in_=x_r[:, gs, 0:512])
            if g == 1:
                # mid-stream so they land before A(0)'s rope drains, without
                # delaying either the first chunks or x(1).
                nc.sync.dma_start(out=cos_sb[:], in_=cos_r[:])
                nc.sync.dma_start(out=sinf_sb[:], in_=sinf_r[:])
        issue_x(1)
        nc.sync.dma_start(out=mask_sb[:], in_=mask_r[:])
        nc.sync.dma_start(out=wo_sb[:], in_=wo_r[:])

        def rope_drain(ps, j, dst):
            """dst = ps*cos + rotate_half(ps)*sin for slice j (sin sign-folded)."""
            raw = tmp.tile([128, 512], F16, tag="raw", name="raw", bufs=5)
            nc.scalar.copy(raw[:], ps[:])
            cs = cos_sb[:, 512 * j:512 * (j + 1)]
            sn = sinf_sb[:, 512 * j:512 * (j + 1)]
            rot = tmp.tile([128, 512], F16, tag="rot", name="rot", bufs=3)
            nc.vector.tensor_mul(dst[:], raw[:], cs)
            # sin's halves are identical, so with the sign fold on the BOTTOM
            # half (sinf[64:] = -sin) both muls read same-base-partition pairs
            # (a birverifier requirement for SB+SB tensor_tensor).
            nc.vector.tensor_mul(rot[0:64, :], raw[64:128, :], sn[64:128, :])
            nc.vector.tensor_mul(rot[64:128, :], raw[0:64, :], sn[0:64, :])
            nc.vector.tensor_add(dst[:], dst[:], rot[:])

        def q_drain(ps, j, f):
            qt = qkp.tile([128, 512], F16, tag=f"qt{f}", name=f"qt{j}{f}")
            qts[(j, f)] = qt
            rope_drain(ps, j, qt)

        def k_drain(ps, j):
            # K slices are read by every later B(j') — persistent, one per j.
            kt = vp.tile([128, 512], F16, tag=f"kt{j}", name=f"kt{j}")
            kts[j] = kt
            rope_drain(ps, j, kt)

        def v_drain(ps, j):
            v = vp.tile([128, 512], F16, tag=f"v{j}", name=f"v{j}")
            vsb[j] = v
            nc.scalar.copy(v[:], ps[:])

        # ---------------- A(0): k-outer so compute starts on chunk 0 --------
        with tc.tile_pool(name="psA0", bufs=1, space="PSUM") as psA0:
            qps = [psA0.tile([128, 512], F32, tag=f"p{f}", name=f"qps{f}")
                   for f in range(NHQ)]
            kps = psA0.tile([128, 512], F32, tag="p4", name="kps")
            vps = psA0.tile([128, 512], F32, tag="p5", name="vps")
            def a0_mms(f, kk):
                rhs = xa[(0, kk // 8)][:, kk % 8, :]
                st, sp = (kk == 0), (kk == KT - 1)
                if f < 4:
                    nc.tensor.matmul(qps[f][:], wq_sb[:, kk, 128 * f:128 * (f + 1)],
                                     rhs, start=st, stop=sp, skip_group_check=True)
                elif f == 4:
                    nc.tensor.matmul(kps[:], wk_sb[:, kk, :], rhs,
                                     start=st, stop=sp, skip_group_check=True)
                else:
                    for i in range(4):
                        # start clears the whole PSUM bank, so only the first
                        # sub-block group may assert it.
                        nc.tensor.matmul(vps[:, 128 * i:128 * (i + 1)],
                                         xa[(0, kk // 8)][:, kk % 8,
                                                          128 * i:128 * (i + 1)],
                                         wv_sb[:, kk, :],
                                         start=(st and i == 0), stop=sp,
                                         skip_group_check=True)

            for kk in range(KT - 12):
                for f in range(6):
                    a0_mms(f, kk)
            # staggered tails: finish+drain K, q0, V first so B(0) can start
            # as soon as the last tail lands (K's rope leads the DVE queue).
            for f in (4, 0, 5, 1, 2, 3):
                for kk in range(KT - 12, KT):
                    a0_mms(f, kk)
                if f < 4:
                    q_drain(qps[f], 0, f)
                elif f == 4:
                    k_drain(kps, 0)
                else:
                    v_drain(vps, 0)

        # Steady-state PSUM pools, created after psA0's 6 banks are released.
        psA = ctx.enter_context(tc.tile_pool(name="psA", bufs=1, space="PSUM"))
        psB = ctx.enter_context(tc.tile_pool(name="psB", bufs=3, space="PSUM"))
        psO = ctx.enter_context(tc.tile_pool(name="psO", bufs=2, space="PSUM"))
        psC = ctx.enter_context(tc.tile_pool(name="psC", bufs=2, space="PSUM"))

        # ---------------- quantum generators for filler ----------------------
        def a_quanta(j):
            """A(j) for j>=1, output-outer: 6 chains x 8 pieces of ~850ns."""
            out = deque()
            state = {}

            def piece_fn(f, piece):
                def run():
                    if piece == 0:
                        state[f] = psA.tile([128, 512], F32, tag="aps",
                                            name=f"aps{j}{f}")
                    ps = state[f]
                    if f < 5:
                        for kl in range(4):
                            kk = 4 * piece + kl
                            w = (wq_sb[:, kk, 128 * f:128 * (f + 1)] if f < 4
                                 else wk_sb[:, kk, :])
                            nc.tensor.matmul(ps[:], w, xa[(j, kk // 8)][:, kk % 8, :],
                                             start=(kk == 0), stop=(kk == KT - 1),
                                             skip_group_check=True)
                    else:
                        for m in range(16):
                            idx = 16 * piece + m
                            kk, i = divmod(idx, 4)
                            nc.tensor.matmul(ps[:, 128 * i:128 * (i + 1)],
                                             xa[(j, kk // 8)][:, kk % 8,
                                                              128 * i:128 * (i + 1)],
                                             wv_sb[:, kk, :],
                                             start=(kk == 0 and i == 0),
                                             stop=(kk == KT - 1),
                                             skip_group_check=True)
                    if piece == 7:
                        if f < 4:
                            q_drain(ps, j, f)
                        elif f == 4:
                            k_drain(ps, j)
                        else:
                            v_drain(ps, j)
                return run

            for f in range(6):
                for piece in range(8):
                    out.append(piece_fn(f, piece))
            return out

        def c_quanta(jc):
            """C(jc): 8 n-slices x 4 sq-subtiles, one ~850ns quantum each."""
            out = deque()
            state = {}

            def group_fn(n, q):
                def run():
                    if q == 0:
                        state[n] = ysp.tile([128, 4, 512], F16, tag="ys",
                                            name=f"ys{jc}{n}")
                    ys = state[n]
                    # during C(3) no A-chain or B work remains, so the idle
                    # psA and psO banks join the yp rotation (5 slots).
                    m = (4 * n + q) % 5
                    if jc == 3 and m == 2:
                        yp = psA.tile([128, 512], F32, tag="aps", name=f"yp{jc}{n}{q}")
                    elif jc == 3 and m >= 3:
                        yp = psO.tile([128, 512], F32, tag="ops", name=f"yp{jc}{n}{q}")
                    else:
                        yp = psC.tile([128, 512], F32, tag="yp", name=f"yp{jc}{n}{q}")
                    for k in range(NHQ):
                        nc.tensor.matmul(yp[:], ots[(jc, k)][:, 128 * q:128 * (q + 1)],
                                         wo_sb[:, k, 512 * n:512 * (n + 1)],
                                         start=(k == 0), stop=(k == NHQ - 1),
                                         skip_group_check=True)
                    if (n + q) % 2 == 0:
                        nc.vector.tensor_copy(ys[:, q, :], yp[:])
                    else:
                        nc.scalar.copy(ys[:, q, :], yp[:])
                    # The very last group's writeback is split so the final DMA
                    # covers only one 128-row tile (shorter kernel tail).
                    if jc == 3 and n == NO - 1:
                        if q == 1:
                            nc.sync.dma_start(
                                out=y[512 * jc:512 * jc + 256, 512 * n:512 * (n + 1)]
                                .rearrange("(q p) m -> p q m", p=128),
                                in_=ys[:, 0:2, :])
                        elif q >= 2:
                            r0 = 512 * jc + 128 * q
                            nc.sync.dma_start(
                                out=y[r0:r0 + 128, 512 * n:512 * (n + 1)],
                                in_=ys[:, q, :])
                    elif q == 3:
                        nc.sync.dma_start(
                            out=y[512 * jc:512 * (jc + 1), 512 * n:512 * (n + 1)]
                            .rearrange("(q p) m -> p q m", p=128),
                            in_=ys[:])
                return run

            if jc == 3:
                # split-emit the first 5 groups: their k=0..2 contractions can
                # run during the last head's normalization-chain latency; the
                # k=3 matmuls (which wait on ot(3,3)) follow.
                ypre = {}

                def pre_fn(n, q, i):
                    def run():
                        if q == 0:
                            state[n] = ysp.tile([128, 4, 512], F16, tag="ys",
                                                name=f"ys{jc}{n}")
                        # five DISTINCT banks so no pre-group waits on another
                        pool, tag = [(psC, "yp"), (psC, "yp"), (psA, "aps"),
                                     (psO, "ops"), (psO, "ops")][i]
                        yp = pool.tile([128, 512], F32, tag=tag,
                                       name=f"yp{jc}{n}{q}")
                        ypre[(n, q)] = yp
                        for k in range(NHQ - 1):
                            nc.tensor.matmul(
                                yp[:], ots[(jc, k)][:, 128 * q:128 * (q + 1)],
                                wo_sb[:, k, 512 * n:512 * (n + 1)],
                                start=(k == 0), stop=False,
                                skip_group_check=True)
                    return run

                def post_fn(n, q):
                    def run():
                        ys = state[n]
                        yp = ypre[(n, q)]
                        k = NHQ - 1
                        nc.tensor.matmul(
                            yp[:], ots[(jc, k)][:, 128 * q:128 * (q + 1)],
                            wo_sb[:, k, 512 * n:512 * (n + 1)],
                            start=False, stop=True, skip_group_check=True)
                        if (n + q) % 2 == 0:
                            nc.vector.tensor_copy(ys[:, q, :], yp[:])
                        else:
                            nc.scalar.copy(ys[:, q, :], yp[:])
                        if q == 3:
                            nc.sync.dma_start(
                                out=y[512 * jc:512 * (jc + 1),
                                      512 * n:512 * (n + 1)]
                                .rearrange("(q p) m -> p q m", p=128),
                                in_=ys[:])
                    return run

                for i in range(5):
                    out.append(pre_fn(i // 4, i % 4, i))
                for i in range(5):
                    out.append(post_fn(i // 4, i % 4))
                for i in range(5, NO * 4):
                    out.append(group_fn(i // 4, i % 4))
            else:
                for n in range(NO):
                    for q in range(4):
                        out.append(group_fn(n, q))
            return out

        fillerA = deque()
        fillerC = deque()

        def pull():
            if fillerC:
                fillerC.popleft()()
            elif fillerA:
                fillerA.popleft()()

        # ---------------- B(j): attention with interleaved filler ------------
        def emit_b(j):
            # Flat (h, t) loop: the pv pipeline (depth 3) carries across head
            # boundaries, so the exp-latency refill bubble is paid once per j
            # instead of once per head.
            tmax = 4 * j + 4
            state = {}
            pend = deque()
            cnt = 0

            def emit_pv(ent):
                h, t, ptb, c0 = ent
                ops, pacc = state[h]
                nc.tensor.matmul(ops[:, c0:512],
                                 vsb[t // 4][:, 128 * (t % 4):128 * (t % 4 + 1)],
                                 ptb[:, c0:512], start=(t == 0),
                                 stop=(t == tmax - 1), skip_group_check=True)
                if t == tmax - 1:
                    lsum = nrmp.tile([128, 512], F32, tag="lsum",
                                     name=f"lsum{j}{h}", bufs=1)
                    nc.gpsimd.partition_all_reduce(
                        lsum[:], pacc[:], channels=128,
                        reduce_op=bass_isa.ReduceOp.add)
                    rb = nrmp.tile([128, 512], F16, tag="rb", name=f"rb{j}{h}",
                                   bufs=1)
                    nc.vector.reciprocal(rb[:], lsum[:])
                    ot = otp.tile([128, 512], F16, tag=f"ot{h}", name=f"ot{j}{h}")
                    ots[(j, h)] = ot
                    nc.vector.tensor_mul(ot[:], ops[:], rb[:])

            for h in range(NHQ):
                for t in range(tmax):
                    if t == 0:
                        state[h] = (
                            psO.tile([128, 512], F32, tag="ops", name=f"ops{j}{h}"),
                            paccp.tile([128, 512], F16, tag="pacc",
                                       name=f"pacc{j}{h}"))
                    ops, pacc = state[h]
                    r = t - 4 * j
                    c0 = 128 * r if r >= 0 else 0
                    scp = psB.tile([128, 512], F32, tag="scp", name=f"scp{j}{h}{t}")
                    nc.tensor.matmul(scp[:, c0:512],
                                     kts[t // 4][:, 128 * (t % 4):128 * (t % 4 + 1)],
                                     qts[(j, h)][:, c0:512], start=True, stop=True,
                                     skip_group_check=True)
                    if r >= 0:
                        nc.vector.tensor_add(scp[:, c0:c0 + 128],
                                             scp[:, c0:c0 + 128], mask_sb[:])
                    ptb = ptbp.tile([128, 512], F16, tag="ptb", name=f"ptb{j}{h}{t}")
                    nc.scalar.activation(ptb[:, c0:512], scp[:, c0:512], EXP,
                                         bias=0.0, scale=SCALE)
                    if t == 0:
                        nc.vector.tensor_copy(pacc[:], ptb[:])
                    else:
                        nc.vector.tensor_add(pacc[:, c0:512], pacc[:, c0:512],
                                             ptb[:, c0:512])
                    pend.append((h, t, ptb, c0))
                    if len(pend) > 8:
                        emit_pv(pend.popleft())
                    cnt += 1
                    if j <= 2:
                        pull()
                        pull()
                    elif cnt % 2 == 0:
                        pull()
            while pend:
                emit_pv(pend.popleft())

        # ---------------- main fused j loop ----------------------------------
        for j in range(SL):
            while fillerA:
                fillerA.popleft()()              # A(j) leftover
            if j + 2 < SL:
                issue_x(j + 2)
            if j + 1 < SL:
                fillerA.extend(a_quanta(j + 1))
            emit_b(j)
            while fillerC:
                fillerC.popleft()()              # C(j-1) leftover
            fillerC.extend(c_quanta(j))
        while fillerA:
            fillerA.popleft()()
        while fillerC:
            fillerC.popleft()()                  # C(3)

    nc.compile()
    return nc


def get_nc():
    if "nc" not in _NC_CACHE:
        _NC_CACHE["nc"] = build_nc()
    return _NC_CACHE["nc"]


def make_in_maps(hidden_states, attention_mask, position_ids, Wq, Wk, Wv, Wo):
    hs = np.asarray(hidden_states, dtype=np.float32)
    pos = np.asarray(position_ids)
    Wq = np.asarray(Wq, dtype=np.float32)
    Wk = np.asarray(Wk, dtype=np.float32)
    Wv = np.asarray(Wv, dtype=np.float32)
    Wo = np.asarray(Wo, dtype=np.float32)
    assert hs.shape == (1, S, HID)
    assert Wq.shape == (HID, HID) and Wk.shape == (HID, 1024)
    assert Wv.shape == (HID, 1024) and Wo.shape == (HID, HID)

    # x_r[p, t, s] = hidden[0, s, 128t+p]
    x_r = np.ascontiguousarray(
        hs[0].T.reshape(KT, 128, S).transpose(1, 0, 2)).astype(np.float16)

    p = pos[0].astype(np.float32)
    inv = (1.0 / (10000.0 ** (np.arange(0, D, 2, dtype=np.float32)
                              / np.float32(D)))).astype(np.float32)
    freqs = p[:, None] * inv[None, :]
    emb = np.concatenate([freqs, freqs], axis=1)        # (S, 128)
    cos_r = np.ascontiguousarray(np.cos(emb).T).astype(np.float16)
    sinT = np.sin(emb).T.astype(np.float32)
    sinT[64:] *= np.float32(-1.0)
    sinf_r = np.ascontiguousarray(sinT).astype(np.float16)

    ar = np.arange(128)
    mask_r = np.where(ar[:, None] <= ar[None, :], np.float32(0),
                      np.float32(-1e9 / SCALE)).astype(np.float32)

    in_maps = []
    for c in range(NCORES):
        wq_c = Wq[:, 512 * c:512 * (c + 1)]
        wk_c = Wk[:, 128 * c:128 * (c + 1)]
        wv_c = Wv[:, 128 * c:128 * (c + 1)]
        wo_c = Wo[512 * c:512 * (c + 1), :]
        w_pack = np.concatenate([wq_c, wk_c, wv_c], axis=1)     # [HID, 768]
        in_maps.append({
            "x_r": x_r,
            "w_r": np.ascontiguousarray(
                w_pack.reshape(KT, 128, 768).transpose(1, 0, 2)).astype(np.float16),
            "wo_r": np.ascontiguousarray(
                wo_c.reshape(NHQ, 128, HID).transpose(1, 0, 2)).astype(np.float16),
            "cos_r": cos_r,
            "sinf_r": sinf_r,
            "mask_r": mask_r,
        })
    return in_maps


def kernel(hidden_states, attention_mask, position_ids, Wq, Wk, Wv, Wo):
    # The axon NTFF trace hook isn't shipped in this container; make sure a
    # stray BASS_TRACE in the environment can't route us onto that path.
    os.environ["BASS_NEVER_TRACE"] = "1"
    in_maps = make_in_maps(hidden_states, attention_mask, position_ids,
                           Wq, Wk, Wv, Wo)
    nc = get_nc()
    res = run_bass_kernel_spmd(nc, in_maps, list(range(NCORES)))
    acc = np.zeros((S, HID), dtype=np.float64)
    for c in range(NCORES):
        acc += res.results[c]["y"]
    return acc.astype(np.float32)[None]
